# revision 1
# baseline (speedup 1.0000x reference)
"""Trainium2 Bass kernel: multi-head self-attention over images (1x1-conv QKV).

Problem: x [4, 256, 64, 64], w_qkv [384, 256], w_out [256, 128], b_out [256].
  qkv = w_qkv @ x_flat ; per-head (h=4, d=32) softmax attention over n=4096 ;
  out = w_out @ heads + b_out.

Sharding across 8 cores: (batch, query-half) pairs -> each core handles one
batch's K/V over the full 4096 positions and attention + output projection
for 2048 of its query positions.  Outputs are disjoint slices; the host
only concatenates.

Per-core dataflow (all fp32):
  - QKV projections on TensorE; K kept as [head*d, j], Q as [head*d, i],
    V produced transposed as [j, head*d] (+ a ones column per head for the
    softmax denominator).
  - simT[j, i] = K^T-tile . Q via K=32 row-tiles.  HW constraint (found
    empirically): temporally-overlapping matmuls must not write the same
    PSUM bank at the same partitions, and same-tile_position matmuls
    serialize.  So heads live in two "planes" at array row positions 0 and
    64 only; the two concurrent heads write different PSUM banks.
  - exp on ScalarE straight from PSUM (N=1024 per instruction) -> SBUF.
    ScalarE is the bottleneck engine (~33.5M exp/core); no max-subtraction
    is needed (sim values are O(1) by construction).
  - AV: out_T[d(+sum), i] accumulates over j-tiles in PSUM with V'^T as the
    stationary operand, 2-way column-tiled (two heads concurrently).
  - Softmax division: row-sums broadcast across partitions with a K=1
    matmul against a constant mask, reciprocal + multiply on VectorE.
  - Output projection with host-prescrambled w_out halves + bias.
"""

import sys

import numpy as np

for _p in ("/opt/trn_rl_repo",):
    if _p not in sys.path:
        sys.path.insert(0, _p)

HEADS = 4
DH = 32
DIM = 256
HID = HEADS * DH  # 128
B = 4
N_CORES = 8
NJ = 4096  # full context per batch (64*64)
NI = NJ // 2  # queries per core
IC = 256  # i-chunk (columns per inner tile)


def build_attn(tc, out_ap, in_aps, nj, ni, ic, repeat=1):
    """Emit the per-core attention program.

    out_ap: DRAM AP [256, ni]
    in_aps: dict with DRAM APs:
        x_kv  [256, nj]   batch image, channels-major
        x_q   [256, ni]   this core's query columns
        w_qkvt [256, 384] (w_qkv with q-scale folded in).T
        w_out_a/w_out_b [128, 256] prescrambled output weights
        b2    [128, 2]    bias, b2[p, mc] = b_out[mc*128 + p]
    """
    import concourse.tile as tile  # noqa: F401
    from concourse import mybir

    nc = tc.nc
    f32 = mybir.dt.float32
    Exp = mybir.ActivationFunctionType.Exp
    jt_n = nj // 128
    nic = ni // ic
    assert nj % 512 == 0 and ni % 512 == 0 and ni % ic == 0

    from contextlib import ExitStack

    with ExitStack() as ctx:
        const = ctx.enter_context(tc.tile_pool(name="const", bufs=1))
        sim_pool = ctx.enter_context(tc.tile_pool(name="simps", bufs=2, space="PSUM"))
        av_pool = ctx.enter_context(tc.tile_pool(name="avps", bufs=1, space="PSUM"))
        work_pool = ctx.enter_context(tc.tile_pool(name="workps", bufs=2, space="PSUM"))
        exp_pool = ctx.enter_context(tc.tile_pool(name="expsb", bufs=3))
        sb_pool = ctx.enter_context(tc.tile_pool(name="sb", bufs=2))

        # ---------------- persistent SBUF ----------------
        x_sb = const.tile([128, 2, nj], f32, tag="x_sb")
        xq_sb = const.tile([128, 2, ni], f32, tag="xq_sb")
        wqkvt_sb = const.tile([128, 2, 5 * HID], f32, tag="wqkvt")
        wouta_sb = const.tile([128, 256], f32, tag="wouta")
        woutb_sb = const.tile([128, 256], f32, tag="woutb")
        b_sb = const.tile([128, 2], f32, tag="b_sb")
        mask_sb = const.tile([128, 128], f32, tag="mask")
        k_sb = const.tile([128, 2, nj], f32, tag="k_sb")
        q_sb = const.tile([128, 2, ni], f32, tag="q_sb")
        vt_sb = const.tile([128, jt_n, HEADS, DH + 1], f32, tag="vt_sb")
        # normalized head outputs; [hd, buf, pair, ic]; rows 33-63/97-127 stay 0
        oh_sb = const.tile([128, 2, 2, ic], f32, tag="oh_sb")

        # ---------------- input DMAs ----------------
        nc.sync.dma_start(x_sb[:], in_aps["x_kv"].rearrange("(c p) n -> p c n", p=128))
        nc.sync.dma_start(xq_sb[:], in_aps["x_q"].rearrange("(c p) n -> p c n", p=128))
        nc.sync.dma_start(
            wqkvt_sb[:], in_aps["w_qkvt"].rearrange("(c p) m -> p c m", p=128)
        )
        nc.sync.dma_start(wouta_sb[:], in_aps["w_out_a"][:])
        nc.sync.dma_start(woutb_sb[:], in_aps["w_out_b"][:])
        nc.sync.dma_start(b_sb[:], in_aps["b2"][:])

        for _rep in range(repeat):
            # ---------------- constants ----------------
            nc.vector.memset(mask_sb[:], 0.0)
            nc.vector.memset(mask_sb[32:33, 0:33], 1.0)
            nc.vector.memset(mask_sb[96:97, 64:97], 1.0)
            nc.vector.memset(vt_sb[:, :, :, DH : DH + 1], 1.0)
            # rows 33-63 / 97-127 stay zero; live rows are rewritten every chunk
            nc.vector.memset(oh_sb[:], 0.0)

            # ---------------- projections ----------------
            # K planes: k_sb[part, plane, j]
            for pl in range(2):
                for jc in range(nj // 512):
                    ps = work_pool.tile([128, 2, 256], f32, tag="work")
                    psf = ps[:].rearrange("p a b -> p (a b)")
                    for cs in range(2):
                        nc.tensor.matmul(
                            psf,
                            lhsT=wqkvt_sb[:, cs, 2 * HID + 128 * pl : 2 * HID + 128 * (pl + 1)],
                            rhs=x_sb[:, cs, jc * 512 : (jc + 1) * 512],
                            start=(cs == 0),
                            stop=(cs == 1),
                        )
                    nc.vector.tensor_copy(k_sb[:, pl, jc * 512 : (jc + 1) * 512], psf)
            # Q planes (pre-scaled via w_qkvt)
            for pl in range(2):
                for qc in range(ni // 512):
                    ps = work_pool.tile([128, 2, 256], f32, tag="work")
                    psf = ps[:].rearrange("p a b -> p (a b)")
                    for cs in range(2):
                        nc.tensor.matmul(
                            psf,
                            lhsT=wqkvt_sb[:, cs, 128 * pl : 128 * (pl + 1)],
                            rhs=xq_sb[:, cs, qc * 512 : (qc + 1) * 512],
                            start=(cs == 0),
                            stop=(cs == 1),
                        )
                    nc.vector.tensor_copy(q_sb[:, pl, qc * 512 : (qc + 1) * 512], psf)
            # V^T: vt_sb[j, h, d] (+ ones column at d=DH)
            for jt in range(jt_n):
                ps = work_pool.tile([128, 2, 256], f32, tag="work")
                psf = ps[:].rearrange("p a b -> p (a b)")
                for cs in range(2):
                    nc.tensor.matmul(
                        psf[:, 0:HID],
                        lhsT=x_sb[:, cs, jt * 128 : (jt + 1) * 128],
                        rhs=wqkvt_sb[:, cs, 4 * HID : 5 * HID],
                        start=(cs == 0),
                        stop=(cs == 1),
                    )
                nc.vector.tensor_copy(
                    vt_sb[:, jt, :, 0:DH],
                    psf[:, 0:HID].rearrange("p (h d) -> p h d", h=HEADS),
                )

            out_r = out_ap.rearrange("(m p) n -> p m n", p=128)

            # ---------------- main loop ----------------
            for ici in range(nic):
                i_sl = slice(ici * ic, (ici + 1) * ic)
                buf = ici % 2
                # one bank per head-pair: start=True clears has_written bank-wide
                av_ps = av_pool.tile([128, 2, 512], f32, tag="av")
                for jt in range(jt_n):
                    sim_ps = sim_pool.tile([128, HEADS, ic], f32, tag="sim")
                    # issue order h0(b0,r0) h1(b1,r64) h2(b0,r0) h3(b1,r64):
                    # concurrent pairs hit different banks; same-bank pairs
                    # share a tile_position and therefore serialize.
                    for h in range(HEADS):
                        pl, sl = h // 2, h % 2
                        col = sl * 2 + pl  # h0->0 h1->2 h2->1 h3->3
                        nc.tensor.matmul(
                            sim_ps[:, col, :],
                            lhsT=k_sb[64 * sl : 64 * sl + 32, pl, jt * 128 : (jt + 1) * 128],
                            rhs=q_sb[64 * sl : 64 * sl + 32, pl, i_sl],
                            start=True,
                            stop=True,
                            tile_position=(64 * sl, 0),
                        )
                    ex = exp_pool.tile([128, HEADS, ic], f32, tag="exp")
                    nc.scalar.activation(ex[:], sim_ps[:], Exp)
                    for h in range(HEADS):
                        pr = h // 2
                        pos = (h % 2) * 64
                        col = (h % 2) * 2 + h // 2
                        # interleaved accumulation groups at disjoint partition
                        # ranges of one bank; the sim's group check is
                        # partition-blind, so it must be skipped (HW-legal).
                        nc.tensor.matmul(
                            av_ps[pos : pos + DH + 1, pr, 0:ic],
                            lhsT=vt_sb[:, jt, h, :],
                            rhs=ex[:, col, :],
                            start=(jt == 0),
                            stop=(jt == jt_n - 1),
                            skip_group_check=True,
                        )

                # ---- softmax division + output projection ----
                sums = sb_pool.tile([128, 2, ic], f32, tag="sums")
                nc.vector.tensor_copy(sums[32:33], av_ps[32:33, :, 0:ic])
                nc.vector.tensor_copy(sums[96:97], av_ps[96:97, :, 0:ic])
                bc_e = work_pool.tile([128, 2, ic], f32, tag="work")
                bc_o = work_pool.tile([128, 2, ic], f32, tag="work")
                for pr in range(2):
                    nc.tensor.matmul(
                        bc_e[:, pr, :],
                        lhsT=mask_sb[32:33, :],
                        rhs=sums[32:33, pr, :],
                        start=True,
                        stop=True,
                        tile_position=(32, 0),
                    )
                    nc.tensor.matmul(
                        bc_o[:, pr, :],
                        lhsT=mask_sb[96:97, :],
                        rhs=sums[96:97, pr, :],
                        start=True,
                        stop=True,
                        tile_position=(96, 0),
                    )
                recip = sb_pool.tile([128, 2, ic], f32, tag="recip")
                nc.vector.reciprocal(recip[0:33], bc_e[0:33])
                nc.vector.reciprocal(recip[64:97], bc_o[64:97])
                nc.vector.tensor_mul(oh_sb[0:33, buf], av_ps[0:33, :, 0:ic], recip[0:33])
                nc.vector.tensor_mul(oh_sb[64:97, buf], av_ps[64:97, :, 0:ic], recip[64:97])

                pout = work_pool.tile([128, 2, ic], f32, tag="work")
                for mc in range(2):
                    nc.tensor.matmul(
                        pout[:, mc, :],
                        lhsT=wouta_sb[:, mc * 128 : (mc + 1) * 128],
                        rhs=oh_sb[:, buf, 0, :],
                        start=True,
                        stop=False,
                    )
                    nc.tensor.matmul(
                        pout[:, mc, :],
                        lhsT=woutb_sb[:, mc * 128 : (mc + 1) * 128],
                        rhs=oh_sb[:, buf, 1, :],
                        start=False,
                        stop=True,
                    )
                final = sb_pool.tile([128, 2, ic], f32, tag="final")
                for mc in range(2):
                    nc.vector.tensor_scalar_add(
                        final[:, mc, :], pout[:, mc, :], b_sb[:, mc : mc + 1]
                    )
                nc.sync.dma_start(out_r[:, :, i_sl], final[:])


def _host_prep(w_qkv, w_out, b_out):
    scale = DH**-0.5
    w_qkv = np.asarray(w_qkv, dtype=np.float32)
    wq = w_qkv[0:HID] * scale
    wk = w_qkv[HID : 2 * HID]
    wv = w_qkv[2 * HID :]
    # planes: plane p holds heads {2p, 2p+1}; within a plane the even slot
    # sits at psum partitions 0-31 (array rows 0-31) and the odd slot at
    # partitions 64-95 (rows 64-95); rows 32-63/96-127 are zero.
    w_qkvt = np.zeros((DIM, 5 * HID), np.float32)  # [256, 640]
    for p in range(2):
        for s_ in range(2):
            h = 2 * p + s_
            w_qkvt[:, 128 * p + 64 * s_ : 128 * p + 64 * s_ + 32] = wq[
                32 * h : 32 * h + 32
            ].T
            w_qkvt[:, 256 + 128 * p + 64 * s_ : 256 + 128 * p + 64 * s_ + 32] = wk[
                32 * h : 32 * h + 32
            ].T
    w_qkvt[:, 4 * HID :] = wv.T
    w_qkvt = np.ascontiguousarray(w_qkvt)
    w_outT = np.asarray(w_out, dtype=np.float32).T  # [128, 256]
    wouta = np.zeros((128, 256), np.float32)
    woutb = np.zeros((128, 256), np.float32)
    wouta[0:32] = w_outT[0:32]  # head 0
    wouta[64:96] = w_outT[32:64]  # head 1
    woutb[0:32] = w_outT[64:96]  # head 2
    woutb[64:96] = w_outT[96:128]  # head 3
    b2 = np.ascontiguousarray(
        np.asarray(b_out, dtype=np.float32).reshape(2, 128).T
    )  # [128, 2]
    return w_qkvt, wouta, woutb, b2


def _build_program(repeat=1):
    import concourse.tile as tile
    from concourse import bacc, mybir

    f32 = mybir.dt.float32
    nc = bacc.Bacc("TRN2", target_bir_lowering=False, debug=False)
    x_kv_h = nc.declare_dram_parameter("x_kv", [DIM, NJ], f32, isOutput=False)
    x_q_h = nc.declare_dram_parameter("x_q", [DIM, NI], f32, isOutput=False)
    w_qkvt_h = nc.declare_dram_parameter("w_qkvt", [DIM, 5 * HID], f32, isOutput=False)
    wouta_h = nc.declare_dram_parameter("w_out_a", [128, 256], f32, isOutput=False)
    woutb_h = nc.declare_dram_parameter("w_out_b", [128, 256], f32, isOutput=False)
    b2_h = nc.declare_dram_parameter("b2", [128, 2], f32, isOutput=False)
    out_h = nc.declare_dram_parameter("out", [DIM, NI], f32, isOutput=True)

    in_aps = {
        "x_kv": x_kv_h[:],
        "x_q": x_q_h[:],
        "w_qkvt": w_qkvt_h[:],
        "w_out_a": wouta_h[:],
        "w_out_b": woutb_h[:],
        "b2": b2_h[:],
    }
    with tile.TileContext(nc) as tc:
        build_attn(tc, out_h[:], in_aps, NJ, NI, IC, repeat=repeat)
    nc.compile()
    return nc


def _make_in_maps(x, w_qkv, w_out, b_out):
    w_qkvt, wouta, woutb, b2 = _host_prep(w_qkv, w_out, b_out)
    xf = np.asarray(x, dtype=np.float32).reshape(B, DIM, NJ)
    in_maps = []
    for c in range(N_CORES):
        b, half = c // 2, c % 2
        in_maps.append(
            {
                "x_kv": np.ascontiguousarray(xf[b]),
                "x_q": np.ascontiguousarray(xf[b][:, half * NI : (half + 1) * NI]),
                "w_qkvt": w_qkvt,
                "w_out_a": wouta,
                "w_out_b": woutb,
                "b2": b2,
            }
        )
    return in_maps


def _assemble(results):
    out_full = np.empty((B, DIM, NJ), np.float32)
    for c in range(N_CORES):
        b, half = c // 2, c % 2
        out_full[b][:, half * NI : (half + 1) * NI] = results[c]["out"]
    return out_full.reshape(B, DIM, 64, 64)


def _run_spmd(x, w_qkv, w_out, b_out, trace=False):
    from concourse.bass_utils import run_bass_kernel_spmd

    nc = _build_program()
    in_maps = _make_in_maps(x, w_qkv, w_out, b_out)
    res = run_bass_kernel_spmd(nc, in_maps, list(range(N_CORES)), trace=trace)
    return _assemble(res.results), res


def kernel(**inputs):
    out, _ = _run_spmd(
        inputs["x"], inputs["w_qkv"], inputs["w_out"], inputs["b_out"]
    )
    return out



# revision 6
# speedup vs baseline: 1.7825x; 1.7825x over previous
"""Trainium2 Bass kernel: multi-head self-attention over images (1x1-conv QKV).

Problem: x [4, 256, 64, 64], w_qkv [384, 256], w_out [256, 128], b_out [256].
  qkv = w_qkv @ x_flat ; per-head (h=4, d=32) softmax attention over n=4096 ;
  out = w_out @ heads + b_out.

Sharding across 8 cores: (batch, query-half) pairs -> each core handles one
batch's K/V over the full 4096 positions and attention + output projection
for 2048 of its query positions.  Outputs are disjoint slices; the host
only concatenates.

Per-core dataflow:
  - All matmul operands are bf16 (4x faster than fp32 on the PE: fp32 costs
    4 cycles/row vs bf16's 1); accumulation stays fp32 in PSUM.  Host
    pre-converts x and the weights to bf16, so DMA volume halves too.
  - QKV projections on TensorE; K kept as [head*d, j], Q as [head*d, i],
    V produced transposed as [j, head*d] (+ a ones column per head for the
    softmax denominator).
  - simT[j, i] = K^T-tile . Q via K=32 row-tiles.  HW constraint (found
    empirically): temporally-overlapping matmuls must not write the same
    PSUM bank at the same partitions, and same-tile_position matmuls
    serialize.  So heads live in two "planes" at array row positions 0 and
    64 only; the two concurrent heads write different PSUM banks.
  - exp on ScalarE straight from PSUM (N=1024 per instruction) -> SBUF in
    bf16.  ScalarE is the bottleneck engine (~33.5M exp/core); no
    max-subtraction is needed (sim values are O(1) by construction).
  - AV: out_T[d(+sum), i] accumulates over j-tiles in PSUM with V'^T as the
    stationary operand, 2-way column-tiled (two heads concurrently).
  - Softmax division: row-sums broadcast across partitions with a K=1
    matmul against a constant mask, reciprocal + multiply on VectorE.
  - Output projection with host-prescrambled w_out halves + bias.
"""

import sys

import numpy as np

for _p in ("/opt/trn_rl_repo",):
    if _p not in sys.path:
        sys.path.insert(0, _p)

import ml_dtypes

BF16 = ml_dtypes.bfloat16

HEADS = 4
DH = 32
DIM = 256
HID = HEADS * DH  # 128
B = 4
N_CORES = 8
NJ = 4096  # full context per batch (64*64)
NI = NJ // 2  # queries per core
IC = 256  # i-chunk (columns per inner tile)


def build_attn(tc, out_ap, in_aps, nj, ni, ic, repeat=1):
    """Emit the per-core attention program.

    out_ap: DRAM AP [256, ni] fp32
    in_aps: dict with DRAM APs:
        x_kv  [256, nj]   bf16 batch image, channels-major
        x_q   [256, ni]   bf16 this core's query columns
        w_qkvt [256, 384] bf16 (w_qkv with q-scale folded in).T
        w_out_a/w_out_b [128, 256] bf16 prescrambled output weights
        b2    [128, 2]    fp32 bias, b2[p, mc] = b_out[mc*128 + p]
    """
    import concourse.tile as tile  # noqa: F401
    from concourse import mybir

    nc = tc.nc
    f32 = mybir.dt.float32
    bf16 = mybir.dt.bfloat16
    Exp = mybir.ActivationFunctionType.Exp
    jt_n = nj // 128
    nic = ni // ic
    assert nj % 512 == 0 and ni % 512 == 0 and ni % ic == 0

    from contextlib import ExitStack

    with ExitStack() as ctx:
        const = ctx.enter_context(tc.tile_pool(name="const", bufs=1))
        sim_pool = ctx.enter_context(tc.tile_pool(name="simps", bufs=2, space="PSUM"))
        av_pool = ctx.enter_context(tc.tile_pool(name="avps", bufs=1, space="PSUM"))
        work_pool = ctx.enter_context(tc.tile_pool(name="workps", bufs=2, space="PSUM"))
        exp_pool = ctx.enter_context(tc.tile_pool(name="expsb", bufs=7))
        sb_pool = ctx.enter_context(tc.tile_pool(name="sb", bufs=2))

        # ---------------- persistent SBUF ----------------
        x_sb = const.tile([128, 2, nj], bf16, tag="x_sb")
        xq_sb = const.tile([128, 2, ni], bf16, tag="xq_sb")
        wqkvt_sb = const.tile([128, 2, 5 * HID], bf16, tag="wqkvt")
        wouta_sb = const.tile([128, 256], bf16, tag="wouta")
        woutb_sb = const.tile([128, 256], bf16, tag="woutb")
        b_sb = const.tile([128, 2], f32, tag="b_sb")
        mask_sb = const.tile([128, 128], bf16, tag="mask")
        k_sb = const.tile([128, 2, nj], bf16, tag="k_sb")
        q_sb = const.tile([128, 2, ni], bf16, tag="q_sb")
        vt_sb = const.tile([128, jt_n, HEADS, DH + 1], bf16, tag="vt_sb")
        # normalized head outputs; [hd, buf, pair, ic]; rows 33-63/97-127 stay 0
        oh_sb = const.tile([128, 2, 2, ic], bf16, tag="oh_sb")

        # ---------------- input DMAs (split + ordered so compute starts
        # early: qkv weights and the first x/xq chunks first) -------------
        xr = in_aps["x_kv"].rearrange("(c p) n -> p c n", p=128)
        xqr = in_aps["x_q"].rearrange("(c p) n -> p c n", p=128)

        def xdma(jc):
            s = slice(jc * 512, (jc + 1) * 512)
            nc.sync.dma_start(x_sb[:, :, s], xr[:, :, s])

        def xqdma(qc):
            s = slice(qc * 512, (qc + 1) * 512)
            nc.sync.dma_start(xq_sb[:, :, s], xqr[:, :, s])

        nc.sync.dma_start(
            wqkvt_sb[:], in_aps["w_qkvt"].rearrange("(c p) m -> p c m", p=128)
        )
        xdma(0)
        xqdma(0)
        nc.sync.dma_start(wouta_sb[:], in_aps["w_out_a"][:])
        nc.sync.dma_start(woutb_sb[:], in_aps["w_out_b"][:])
        nc.sync.dma_start(b_sb[:], in_aps["b2"][:])
        for jc in range(1, nj // 512):
            xdma(jc)
        for qc in range(1, ni // 512):
            xqdma(qc)

        out_r = out_ap.rearrange("(m p) n -> p m n", p=128)

        # ---------------- emission helpers ----------------
        def kproj(pl, jc):
            ps = work_pool.tile([128, 2, 256], f32, tag="work")
            psf = ps[:].rearrange("p a b -> p (a b)")
            for cs in range(2):
                nc.tensor.matmul(
                    psf,
                    lhsT=wqkvt_sb[:, cs, 2 * HID + 128 * pl : 2 * HID + 128 * (pl + 1)],
                    rhs=x_sb[:, cs, jc * 512 : (jc + 1) * 512],
                    start=(cs == 0),
                    stop=(cs == 1),
                )
            nc.vector.tensor_copy(k_sb[:, pl, jc * 512 : (jc + 1) * 512], psf)

        def qproj(pl, qc):
            ps = work_pool.tile([128, 2, 256], f32, tag="work")
            psf = ps[:].rearrange("p a b -> p (a b)")
            for cs in range(2):
                nc.tensor.matmul(
                    psf,
                    lhsT=wqkvt_sb[:, cs, 128 * pl : 128 * (pl + 1)],
                    rhs=xq_sb[:, cs, qc * 512 : (qc + 1) * 512],
                    start=(cs == 0),
                    stop=(cs == 1),
                )
            nc.vector.tensor_copy(q_sb[:, pl, qc * 512 : (qc + 1) * 512], psf)

        def vproj(jt):
            # V^T: vt_sb[j, h, d] (+ ones column at d=DH)
            ps = work_pool.tile([128, 2, 256], f32, tag="work")
            psf = ps[:].rearrange("p a b -> p (a b)")
            for cs in range(2):
                nc.tensor.matmul(
                    psf[:, 0:HID],
                    lhsT=x_sb[:, cs, jt * 128 : (jt + 1) * 128],
                    rhs=wqkvt_sb[:, cs, 4 * HID : 5 * HID],
                    start=(cs == 0),
                    stop=(cs == 1),
                )
            nc.vector.tensor_copy(
                vt_sb[:, jt, :, 0:DH],
                psf[:, 0:HID].rearrange("p (h d) -> p h d", h=HEADS),
            )

        def sim_exp(av_state, jt):
            """sim matmuls + exp for one j-tile; returns the exp tile."""
            av_ps, ici = av_state
            i_sl = slice(ici * ic, (ici + 1) * ic)
            sim_ps = sim_pool.tile([128, HEADS, ic], f32, tag="sim")
            # issue order h0(b0,r0) h1(b1,r64) h2(b0,r0) h3(b1,r64):
            # concurrent pairs hit different banks; same-bank pairs
            # share a tile_position and therefore serialize.
            for h in range(HEADS):
                pl, sl = h // 2, h % 2
                col = sl * 2 + pl  # h0->0 h1->2 h2->1 h3->3
                nc.tensor.matmul(
                    sim_ps[:, col, :],
                    lhsT=k_sb[64 * sl : 64 * sl + 32, pl, jt * 128 : (jt + 1) * 128],
                    rhs=q_sb[64 * sl : 64 * sl + 32, pl, i_sl],
                    start=True,
                    stop=True,
                    tile_position=(64 * sl, 0),
                )
            ex = exp_pool.tile([128, HEADS, ic], bf16, tag="exp")
            nc.scalar.activation(ex[:], sim_ps[:], Exp)
            return ex

        def av_mms(av_state, jt, ex):
            av_ps, _ = av_state
            for h in range(HEADS):
                pr = h // 2
                pos = (h % 2) * 64
                col = (h % 2) * 2 + h // 2
                # interleaved accumulation groups at disjoint partition
                # ranges of one bank; the sim's group check is
                # partition-blind, so it must be skipped (HW-legal).
                nc.tensor.matmul(
                    av_ps[pos : pos + DH + 1, pr, 0:ic],
                    lhsT=vt_sb[:, jt, h, :],
                    rhs=ex[:, col, :],
                    start=(jt == 0),
                    stop=(jt == jt_n - 1),
                    skip_group_check=True,
                )

        def epilogue_div(av_state):
            """Softmax division: av -> normalized oh_sb (frees the av bank)."""
            av_ps, ici = av_state
            buf = ici % 2
            sums = sb_pool.tile([128, 2, ic], bf16, tag="sums")
            nc.vector.tensor_copy(sums[32:33], av_ps[32:33, :, 0:ic])
            nc.vector.tensor_copy(sums[96:97], av_ps[96:97, :, 0:ic])
            bc_e = work_pool.tile([128, 2, ic], f32, tag="work")
            bc_o = work_pool.tile([128, 2, ic], f32, tag="work")
            for pr in range(2):
                nc.tensor.matmul(
                    bc_e[:, pr, :],
                    lhsT=mask_sb[32:33, :],
                    rhs=sums[32:33, pr, :],
                    start=True,
                    stop=True,
                    tile_position=(32, 0),
                )
                nc.tensor.matmul(
                    bc_o[:, pr, :],
                    lhsT=mask_sb[96:97, :],
                    rhs=sums[96:97, pr, :],
                    start=True,
                    stop=True,
                    tile_position=(96, 0),
                )
            recip = sb_pool.tile([128, 2, ic], f32, tag="recip")
            nc.vector.reciprocal(recip[0:33], bc_e[0:33])
            nc.vector.reciprocal(recip[64:97], bc_o[64:97])
            nc.vector.tensor_mul(oh_sb[0:33, buf], av_ps[0:33, :, 0:ic], recip[0:33])
            nc.vector.tensor_mul(oh_sb[64:97, buf], av_ps[64:97, :, 0:ic], recip[64:97])

        def epilogue_out(ici):
            """Output projection + bias + store for chunk ici."""
            buf = ici % 2
            i_sl = slice(ici * ic, (ici + 1) * ic)
            pout = work_pool.tile([128, 2, ic], f32, tag="work")
            for mc in range(2):
                nc.tensor.matmul(
                    pout[:, mc, :],
                    lhsT=wouta_sb[:, mc * 128 : (mc + 1) * 128],
                    rhs=oh_sb[:, buf, 0, :],
                    start=True,
                    stop=False,
                )
                nc.tensor.matmul(
                    pout[:, mc, :],
                    lhsT=woutb_sb[:, mc * 128 : (mc + 1) * 128],
                    rhs=oh_sb[:, buf, 1, :],
                    start=False,
                    stop=True,
                )
            final = sb_pool.tile([128, 2, ic], f32, tag="final")
            for mc in range(2):
                nc.vector.tensor_scalar_add(
                    final[:, mc, :], pout[:, mc, :], b_sb[:, mc : mc + 1]
                )
            nc.sync.dma_start(out_r[:, :, i_sl], final[:])

        DEFER = 4  # j-tiles of AV deferred past the previous chunk's division

        for _rep in range(repeat):
            # ---------------- constants ----------------
            nc.vector.memset(mask_sb[:], 0.0)
            nc.vector.memset(mask_sb[32:33, 0:33], 1.0)
            nc.vector.memset(mask_sb[96:97, 64:97], 1.0)
            nc.vector.memset(vt_sb[:, :, :, DH : DH + 1], 1.0)
            # rows 33-63 / 97-127 stay zero; live rows are rewritten every chunk
            nc.vector.memset(oh_sb[:], 0.0)

            # minimal prologue: K and Q for the first j/i chunks only; the
            # rest is interleaved into chunk 0 so ScalarE starts ~3us in.
            kproj(0, 0)
            kproj(1, 0)
            qproj(0, 0)
            qproj(1, 0)

            # ---------------- main loop ----------------
            prev = None  # av_state of the previous chunk, division pending
            for ici in range(nic):
                av_ps = av_pool.tile([128, 2, 512], f32, tag="av")
                av_state = (av_ps, ici)
                pending = []
                for jt in range(jt_n):
                    if ici == 0:
                        if jt % 4 == 0 and jt > 0:
                            kproj(0, jt // 4)
                            kproj(1, jt // 4)
                        if jt in (8, 16, 24):
                            qproj(0, jt // 8)
                            qproj(1, jt // 8)
                    if prev is not None and jt == DEFER:
                        epilogue_div(prev)
                        for pjt, pex in pending:
                            av_mms(av_state, pjt, pex)
                        pending = []
                    if prev is not None and jt == DEFER + 2:
                        epilogue_out(prev[1])
                        prev = None
                    ex = sim_exp(av_state, jt)
                    if ici == 0:
                        vproj(jt)
                    if prev is not None and jt < DEFER:
                        pending.append((jt, ex))
                    else:
                        av_mms(av_state, jt, ex)
                prev = av_state
            epilogue_div(prev)
            epilogue_out(prev[1])


def _host_prep(w_qkv, w_out, b_out):
    scale = DH**-0.5
    w_qkv = np.asarray(w_qkv, dtype=np.float32)
    wq = w_qkv[0:HID] * scale
    wk = w_qkv[HID : 2 * HID]
    wv = w_qkv[2 * HID :]
    # planes: plane p holds heads {2p, 2p+1}; within a plane the even slot
    # sits at psum partitions 0-31 (array rows 0-31) and the odd slot at
    # partitions 64-95 (rows 64-95); rows 32-63/96-127 are zero.
    w_qkvt = np.zeros((DIM, 5 * HID), np.float32)  # [256, 640]
    for p in range(2):
        for s_ in range(2):
            h = 2 * p + s_
            w_qkvt[:, 128 * p + 64 * s_ : 128 * p + 64 * s_ + 32] = wq[
                32 * h : 32 * h + 32
            ].T
            w_qkvt[:, 256 + 128 * p + 64 * s_ : 256 + 128 * p + 64 * s_ + 32] = wk[
                32 * h : 32 * h + 32
            ].T
    w_qkvt[:, 4 * HID :] = wv.T
    w_qkvt = np.ascontiguousarray(w_qkvt).astype(BF16)
    w_outT = np.asarray(w_out, dtype=np.float32).T  # [128, 256]
    wouta = np.zeros((128, 256), np.float32)
    woutb = np.zeros((128, 256), np.float32)
    wouta[0:32] = w_outT[0:32]  # head 0
    wouta[64:96] = w_outT[32:64]  # head 1
    woutb[0:32] = w_outT[64:96]  # head 2
    woutb[64:96] = w_outT[96:128]  # head 3
    b2 = np.ascontiguousarray(
        np.asarray(b_out, dtype=np.float32).reshape(2, 128).T
    )  # [128, 2]
    return w_qkvt, wouta.astype(BF16), woutb.astype(BF16), b2


def _build_program(repeat=1):
    import concourse.tile as tile
    from concourse import bacc, mybir

    f32 = mybir.dt.float32
    bf16 = mybir.dt.bfloat16
    nc = bacc.Bacc("TRN2", target_bir_lowering=False, debug=False)
    x_kv_h = nc.declare_dram_parameter("x_kv", [DIM, NJ], bf16, isOutput=False)
    x_q_h = nc.declare_dram_parameter("x_q", [DIM, NI], bf16, isOutput=False)
    w_qkvt_h = nc.declare_dram_parameter("w_qkvt", [DIM, 5 * HID], bf16, isOutput=False)
    wouta_h = nc.declare_dram_parameter("w_out_a", [128, 256], bf16, isOutput=False)
    woutb_h = nc.declare_dram_parameter("w_out_b", [128, 256], bf16, isOutput=False)
    b2_h = nc.declare_dram_parameter("b2", [128, 2], f32, isOutput=False)
    out_h = nc.declare_dram_parameter("out", [DIM, NI], f32, isOutput=True)

    in_aps = {
        "x_kv": x_kv_h[:],
        "x_q": x_q_h[:],
        "w_qkvt": w_qkvt_h[:],
        "w_out_a": wouta_h[:],
        "w_out_b": woutb_h[:],
        "b2": b2_h[:],
    }
    with tile.TileContext(nc) as tc:
        build_attn(tc, out_h[:], in_aps, NJ, NI, IC, repeat=repeat)
    nc.compile()
    return nc


def _make_in_maps(x, w_qkv, w_out, b_out):
    w_qkvt, wouta, woutb, b2 = _host_prep(w_qkv, w_out, b_out)
    xf = np.asarray(x, dtype=np.float32).reshape(B, DIM, NJ).astype(BF16)
    in_maps = []
    for c in range(N_CORES):
        b, half = c // 2, c % 2
        in_maps.append(
            {
                "x_kv": np.ascontiguousarray(xf[b]),
                "x_q": np.ascontiguousarray(xf[b][:, half * NI : (half + 1) * NI]),
                "w_qkvt": w_qkvt,
                "w_out_a": wouta,
                "w_out_b": woutb,
                "b2": b2,
            }
        )
    return in_maps


def _assemble(results):
    out_full = np.empty((B, DIM, NJ), np.float32)
    for c in range(N_CORES):
        b, half = c // 2, c % 2
        out_full[b][:, half * NI : (half + 1) * NI] = results[c]["out"]
    return out_full.reshape(B, DIM, 64, 64)


def _run_spmd(x, w_qkv, w_out, b_out, trace=False):
    from concourse.bass_utils import run_bass_kernel_spmd

    nc = _build_program()
    in_maps = _make_in_maps(x, w_qkv, w_out, b_out)
    res = run_bass_kernel_spmd(nc, in_maps, list(range(N_CORES)), trace=trace)
    return _assemble(res.results), res


def kernel(**inputs):
    out, _ = _run_spmd(
        inputs["x"], inputs["w_qkv"], inputs["w_out"], inputs["b_out"]
    )
    return out
